# revision 1
# baseline (speedup 1.0000x reference)
"""Multi-head GAT layer on 8 Trainium2 NeuronCores (Bass/Tile).

Strategy (v2):
- Nodes sharded 6272/core (padded N=50176, 392 tiles of 128). Each core owns
  the edges whose dst is in its shard.
- P1: every core builds a full node table in its DRAM: row = [wv 384 | el 3 |
  er 3] bf16 (780 B). wv = h @ W_node.T, el/er = attention logit halves.
  One bf16 matmul + one copy per 128-node tile.
- P2: the core's nodes are grouped into 49 windows of 128 dst slots
  (degree-sorted so slots in a window have similar degree). Slot = partition.
  Per window: ONE multi-column indirect DMA gathers wv[src] rows for all
  edges (+1 column for the slot's own row, giving er[dst]); edge weights
  ew = exp(leaky_relu(el+er)) computed vectorized; num/denom accumulated
  with per-column FMA on the vector engine (bf16); then the 384->128
  projection via PE, residual + LayerNorm + relu inline.
- Padding edges gather a dedicated pad row with el=-300 so ew ~= 0.
"""
import sys, os, types, ctypes, contextlib

if '/opt/trn_rl_repo' not in sys.path:
    sys.path.insert(0, '/opt/trn_rl_repo')


def _install_profile_hook():
    try:
        import antenv.axon_hooks  # noqa
        return
    except ImportError:
        pass
    try:
        import antenv
    except ImportError:
        return
    so_path = "/opt/axon/libaxon_pjrt.so"
    hook = None
    if os.path.exists(so_path):
        lib = ctypes.CDLL(so_path)
        if hasattr(lib, "axon_start_nrt_profile"):
            lib.axon_start_nrt_profile.argtypes = [ctypes.POINTER(ctypes.c_int64), ctypes.c_size_t]
            lib.axon_start_nrt_profile.restype = ctypes.c_int64
            lib.axon_stop_nrt_profile.argtypes = [ctypes.c_char_p]
            lib.axon_stop_nrt_profile.restype = ctypes.c_int64

            @contextlib.contextmanager
            def _hook(output_dir, device_ids):
                import jax
                jax.devices()
                if device_ids:
                    ids = (ctypes.c_int64 * len(device_ids))(*device_ids)
                    rc = lib.axon_start_nrt_profile(ids, len(device_ids))
                else:
                    rc = lib.axon_start_nrt_profile(None, 0)
                if rc != 0:
                    raise RuntimeError(f"axon_start_nrt_profile rc={rc}")
                try:
                    yield
                finally:
                    n = lib.axon_stop_nrt_profile(str(output_dir).encode())
                    print(f"ntff profile: {n} file(s) -> {output_dir}", file=sys.stderr)
            hook = _hook
    mod = types.ModuleType("antenv.axon_hooks")
    state = {"hook": hook}
    mod.set_axon_ntff_profile_hook = lambda h: state.__setitem__("hook", h)
    mod.get_axon_ntff_profile_hook = lambda: state["hook"]
    sys.modules["antenv.axon_hooks"] = mod
    antenv.axon_hooks = mod


_install_profile_hook()

import numpy as np
import ml_dtypes
from concourse import bass, bacc, mybir, tile
from concourse.bass_utils import run_bass_kernel_spmd

N_NODES = 50000
F = 128
H = 3
NCORES = 8
NP = 50176                   # padded nodes (392 tiles)
NT = NP // 128               # 392 tiles
NPC = NP // NCORES           # 6272 nodes per core
NWIN = NPC // 128            # 49 windows per core
PADROW = NP                  # pad row block at table rows [NP, NP+128)
ROW = 390                    # table row: wv 384 | el 3 | er 3 (bf16)
NEG_SLOPE = 0.2
LN_EPS = 1e-5

f32 = mybir.dt.float32
bf16 = mybir.dt.bfloat16
i32 = mybir.dt.int32

_PROGRAM_CACHE = {}


def _build_program(CS, SI_TOT, add_node_bias, add_out_bias, gamma_one, beta_zero):
    nc = bacc.Bacc("TRN2", target_bir_lowering=False, debug=False,
                   enable_asserts=False, num_devices=NCORES,
                   dynamic_dma_scratch_size=65536)

    hT_in = nc.dram_tensor("hT", [F, NP], bf16, kind="ExternalInput").ap()
    hwin_in = nc.dram_tensor("hwin", [NPC, F], f32, kind="ExternalInput").ap()
    wcomb_in = nc.dram_tensor("wcomb", [F, ROW], bf16, kind="ExternalInput").ap()
    wsc_in = nc.dram_tensor("wsc", [H * F, F], bf16, kind="ExternalInput").ap()
    ident_in = nc.dram_tensor("ident", [128, 128], f32, kind="ExternalInput").ap()
    wint_in = nc.dram_tensor("wint", [128, SI_TOT], i32, kind="ExternalInput").ap()
    padc_in = nc.dram_tensor("padc", [128, ROW], bf16, kind="ExternalInput").ap()
    brow_in = gam_in = bet_in = bia_in = None
    if add_node_bias:
        brow_in = nc.dram_tensor("brow", [128, ROW], f32, kind="ExternalInput").ap()
    if not gamma_one:
        gam_in = nc.dram_tensor("gam", [128, F], f32, kind="ExternalInput").ap()
    if not beta_zero:
        bet_in = nc.dram_tensor("bet", [128, F], f32, kind="ExternalInput").ap()
    if add_out_bias:
        bia_in = nc.dram_tensor("bia", [128, F], f32, kind="ExternalInput").ap()

    table = nc.dram_tensor("table", [NP + 128, ROW], bf16).ap()
    outy = nc.dram_tensor("outy", [NPC, F], f32, kind="ExternalOutput").ap()

    inv_f = 1.0 / F

    with tile.TileContext(nc) as tc:
        with tc.tile_pool(name="const", bufs=1) as cpool:
            wcomb_t = cpool.tile([128, ROW], bf16)
            nc.sync.dma_start(wcomb_t[:], wcomb_in[:])
            ident_t = cpool.tile([128, 128], f32)
            nc.sync.dma_start(ident_t[:], ident_in[:])
            wsc_c = []
            for c in range(H):
                t = cpool.tile([128, F], bf16, tag=f"wsc{c}")
                nc.sync.dma_start(t[:], wsc_in[c * 128:(c + 1) * 128, :])
                wsc_c.append(t)
            brow_t = gam_t = bet_t = bia_t = None
            if add_node_bias:
                brow_t = cpool.tile([128, ROW], f32)
                nc.sync.dma_start(brow_t[:], brow_in[:])
            if not gamma_one:
                gam_t = cpool.tile([128, F], f32)
                nc.sync.dma_start(gam_t[:], gam_in[:])
            if not beta_zero:
                bet_t = cpool.tile([128, F], f32)
                nc.sync.dma_start(bet_t[:], bet_in[:])
            if add_out_bias:
                bia_t = cpool.tile([128, F], f32)
                nc.sync.dma_start(bia_t[:], bia_in[:])

            # preload every window's gather indices up front so the
            # Pool-engine gathers never queue behind in-window stores
            wi_tiles = []
            _off = 0
            for _w in range(NWIN):
                _C = CS[_w]
                _t = cpool.tile([128, _C + 1], i32, tag=f"wi{_w}")
                nc.sync.dma_start(_t[:], wint_in[:, _off:_off + _C + 1])
                wi_tiles.append(_t)
                _off += _C + 1

            # ---- P1: build the node table -------------------------------
            with (
                tc.tile_pool(name="lhs", bufs=3) as lpool,
                tc.tile_pool(name="row", bufs=4) as rpool,
                tc.tile_pool(name="ps1", bufs=4, space="PSUM") as pspool,
            ):
                padrow = rpool.tile([128, ROW], bf16, tag="pad")
                nc.sync.dma_start(padrow[:], padc_in[:])
                nc.sync.dma_start(table[NP:NP + 128, :], padrow[:])

                for tq in range(NT // 4):
                    lhs4 = lpool.tile([128, 512], bf16, tag="lhs4")
                    nc.sync.dma_start(lhs4[:], hT_in[:, tq * 512:(tq + 1) * 512])
                    for j in range(4):
                        t = tq * 4 + j
                        ps = pspool.tile([128, ROW], f32, tag="ps")
                        nc.tensor.matmul(out=ps[:], lhsT=lhs4[:, j * 128:(j + 1) * 128],
                                         rhs=wcomb_t[:], start=True, stop=True)
                        row = rpool.tile([128, ROW], bf16, tag="row")
                        if add_node_bias:
                            nc.vector.scalar_tensor_tensor(
                                out=row[:], in0=ps[:], scalar=1.0, in1=brow_t[:],
                                op0=mybir.AluOpType.bypass, op1=mybir.AluOpType.add)
                        elif t % 2 == 0:
                            nc.scalar.activation(row[:], ps[:],
                                                 mybir.ActivationFunctionType.Copy)
                        else:
                            nc.vector.tensor_copy(row[:], ps[:])
                        nc.sync.dma_start(table[t * 128:(t + 1) * 128, :], row[:])

            # ---- P2: edge windows --------------------------------------
            with (
                tc.tile_pool(name="gath", bufs=2) as gpool,
                tc.tile_pool(name="edge", bufs=2) as epool,
                tc.tile_pool(name="small", bufs=3) as spool,
                tc.tile_pool(name="accp", bufs=2) as apool,
                tc.tile_pool(name="fin", bufs=3) as fpool,
                tc.tile_pool(name="pst", bufs=2, space="PSUM") as pst,
                tc.tile_pool(name="psp", bufs=2, space="PSUM") as psp,
            ):
                off = 0
                for w in range(NWIN):
                    C = CS[w]
                    agg = apool.tile([128, H * F], f32, tag="agg")
                    if C > 0:
                        gv = gpool.tile([128, C + 1, ROW], bf16, tag="gv")
                        # multi-column offset APs are broken in the indirect
                        # DMA ucode -- issue one [128,1]-offset gather per
                        # column (baseline-proven construct).
                        wi = wi_tiles[w]
                        for j in range(C + 1):
                            nc.gpsimd.indirect_dma_start(
                                out=gv[:, j, :], out_offset=None,
                                in_=table[:],
                                in_offset=bass.IndirectOffsetOnAxis(
                                    ap=wi[:, j:j + 1], axis=0))
                        erw = spool.tile([128, 3], f32, tag="erw")
                        nc.vector.tensor_copy(erw[:], gv[:, C, 387:390])
                        attn = epool.tile([128, 3, C], f32, tag="attn")
                        nc.vector.tensor_tensor(
                            out=attn[:],
                            in0=gv[:, 0:C, 384:387].rearrange("p c h -> p h c"),
                            in1=erw[:, :, None].to_broadcast([128, 3, C]),
                            op=mybir.AluOpType.add)
                        att2 = epool.tile([128, 3, C], f32, tag="att2")
                        nc.vector.scalar_tensor_tensor(
                            out=att2[:], in0=attn[:], scalar=NEG_SLOPE, in1=attn[:],
                            op0=mybir.AluOpType.mult, op1=mybir.AluOpType.max)
                        ew = epool.tile([128, 3, C], f32, tag="ew")
                        nc.scalar.activation(ew[:], att2[:],
                                             mybir.ActivationFunctionType.Exp)
                        ewf = ew[:].rearrange("p a b -> p (a b)")
                        den = spool.tile([128, 3, 1], f32, tag="den")
                        nc.vector.reduce_sum(den[:], ew[:], axis=mybir.AxisListType.X)
                        dmx = spool.tile([128, 3], f32, tag="dmx")
                        nc.vector.tensor_scalar(
                            out=dmx[:], in0=den[:].rearrange("p a b -> p (a b)"),
                            scalar1=1e-9, scalar2=None, op0=mybir.AluOpType.max)
                        dr = spool.tile([128, 3], f32, tag="dr")
                        nc.vector.reciprocal(dr[:], dmx[:])

                        acc = apool.tile([128, H * F], bf16, tag="acc")
                        for c in range(H):
                            nc.vector.tensor_scalar_mul(
                                acc[:, c * 128:(c + 1) * 128],
                                gv[:, 0, c * 128:(c + 1) * 128],
                                ewf[:, c * C:c * C + 1])
                        for j in range(1, C):
                            for c in range(H):
                                nc.vector.scalar_tensor_tensor(
                                    out=acc[:, c * 128:(c + 1) * 128],
                                    in0=gv[:, j, c * 128:(c + 1) * 128],
                                    scalar=ewf[:, c * C + j:c * C + j + 1],
                                    in1=acc[:, c * 128:(c + 1) * 128],
                                    op0=mybir.AluOpType.mult,
                                    op1=mybir.AluOpType.add)
                        for c in range(H):
                            nc.vector.tensor_scalar_mul(
                                agg[:, c * 128:(c + 1) * 128],
                                acc[:, c * 128:(c + 1) * 128],
                                dr[:, c:c + 1])
                    else:
                        nc.vector.memset(agg[:], 0.0)
                    off += C + 1

                    pp = psp.tile([128, F], f32, tag="pp")
                    for c in range(H):
                        tp = pst.tile([128, 128], f32, tag="tp")
                        nc.tensor.transpose(out=tp[:], in_=agg[:, c * 128:(c + 1) * 128],
                                            identity=ident_t[:])
                        aggT = epool.tile([128, 128], bf16, tag="aggT")
                        nc.scalar.activation(aggT[:], tp[:],
                                             mybir.ActivationFunctionType.Copy)
                        nc.tensor.matmul(out=pp[:], lhsT=aggT[:], rhs=wsc_c[c][:],
                                         start=(c == 0), stop=(c == H - 1))

                    hrow = fpool.tile([128, F], f32, tag="hrow")
                    nc.scalar.dma_start(hrow[:], hwin_in[w * 128:(w + 1) * 128, :])
                    x = fpool.tile([128, F], f32, tag="x")
                    nc.vector.tensor_tensor(out=x[:], in0=pp[:], in1=hrow[:],
                                            op=mybir.AluOpType.add)
                    if add_out_bias:
                        x2 = fpool.tile([128, F], f32, tag="x2")
                        nc.vector.tensor_tensor(out=x2[:], in0=x[:], in1=bia_t[:],
                                                op=mybir.AluOpType.add)
                        x = x2
                    # LayerNorm + relu
                    sm = fpool.tile([128, 1], f32, tag="sm")
                    nc.vector.reduce_sum(sm[:], x[:], axis=mybir.AxisListType.X)
                    mu = fpool.tile([128, 1], f32, tag="mu")
                    nc.vector.tensor_scalar_mul(mu[:], sm[:], inv_f)
                    xc = fpool.tile([128, F], f32, tag="xc")
                    nc.vector.tensor_scalar(out=xc[:], in0=x[:], scalar1=mu[:, :1],
                                            scalar2=None, op0=mybir.AluOpType.subtract)
                    sq = fpool.tile([128, F], f32, tag="sq")
                    nc.vector.tensor_tensor(out=sq[:], in0=xc[:], in1=xc[:],
                                            op=mybir.AluOpType.mult)
                    vs = fpool.tile([128, 1], f32, tag="vs")
                    nc.vector.reduce_sum(vs[:], sq[:], axis=mybir.AxisListType.X)
                    vp = fpool.tile([128, 1], f32, tag="vp")
                    nc.vector.tensor_scalar(out=vp[:], in0=vs[:], scalar1=inv_f,
                                            scalar2=LN_EPS, op0=mybir.AluOpType.mult,
                                            op1=mybir.AluOpType.add)
                    rv = fpool.tile([128, 1], f32, tag="rv")
                    nc.vector.reciprocal(rv[:], vp[:])
                    si = fpool.tile([128, 1], f32, tag="si")
                    nc.scalar.activation(si[:], rv[:], mybir.ActivationFunctionType.Sqrt)
                    if gamma_one:
                        y2 = fpool.tile([128, F], f32, tag="y2")
                        nc.vector.tensor_scalar_mul(y2[:], xc[:], si[:, :1])
                    else:
                        y2 = fpool.tile([128, F], f32, tag="y2")
                        nc.vector.scalar_tensor_tensor(
                            out=y2[:], in0=xc[:], scalar=si[:, :1], in1=gam_t[:],
                            op0=mybir.AluOpType.mult, op1=mybir.AluOpType.mult)
                    if not beta_zero:
                        y3 = fpool.tile([128, F], f32, tag="y3")
                        nc.vector.tensor_tensor(out=y3[:], in0=y2[:], in1=bet_t[:],
                                                op=mybir.AluOpType.add)
                        y2 = y3
                    y4 = fpool.tile([128, F], f32, tag="y4")
                    nc.vector.tensor_scalar(out=y4[:], in0=y2[:], scalar1=0.0,
                                            scalar2=None, op0=mybir.AluOpType.max)
                    nc.sync.dma_start(outy[w * 128:(w + 1) * 128, :], y4[:])

    nc.compile()
    return nc


def _pad_row_const():
    pr = np.zeros((128, ROW), np.float32)
    pr[:, 384:387] = -300.0
    return pr.astype(ml_dtypes.bfloat16)


def _host_prep(h, src, dst, W_node, b_node, att, w_scale, bias, ln_gamma, ln_beta):
    src = np.asarray(src).astype(np.int64)
    dst = np.asarray(dst).astype(np.int64)
    h = np.asarray(h, np.float32)
    E = src.shape[0]

    deg = np.bincount(dst, minlength=NP)

    # window assignment: per core, degree-sorted nodes -> windows of 128;
    # window lists sorted by column count so sizes align across cores.
    nodeid = np.zeros((NCORES, NWIN, 128), np.int64)
    CS_k = np.zeros((NCORES, NWIN), np.int64)
    for k in range(NCORES):
        nodes = np.arange(k * NPC, (k + 1) * NPC)
        order = np.argsort(-deg[nodes], kind="stable")
        ns = nodes[order]
        wm = ns.reshape(NWIN, 128)
        cw = deg[wm].max(axis=1)
        worder = np.argsort(-cw, kind="stable")
        nodeid[k] = wm[worder]
        CS_k[k] = cw[worder]
    CS = tuple(int(x) for x in CS_k.max(axis=0))
    SI_TOT = int(sum(c + 1 for c in CS))

    # slot/window of each node, per core
    slot_flat = np.zeros(NP, np.int64)   # w*128+p within its core
    for k in range(NCORES):
        slot_flat[nodeid[k].ravel()] = np.arange(NWIN * 128)

    # per-edge rank within its dst node
    order = np.argsort(dst, kind="stable")
    so = src[order]
    do = dst[order]
    starts = np.zeros(NP, np.int64)
    starts[1:] = np.cumsum(deg)[:-1]
    rank = np.arange(E) - starts[do]

    # build widx per core: [NWIN*128, maxC+1] then slice per window
    offs = np.zeros(NWIN + 1, np.int64)
    for w in range(NWIN):
        offs[w + 1] = offs[w] + CS[w] + 1
    wint = np.full((NCORES, 128, SI_TOT), PADROW, np.int32)
    core_of = do // NPC
    wp = slot_flat[do]          # w*128+p
    wn = wp // 128
    pn = wp % 128
    col = offs[wn] + rank
    csarr = np.asarray(CS, np.int64)
    wnk = np.repeat(np.arange(NWIN), 128)
    pnk = np.tile(np.arange(128), NWIN)
    for k in range(NCORES):
        m = core_of == k
        wint[k, pn[m], col[m]] = so[m]
        # own-node (er) column
        wint[k, pnk, offs[wnk] + csarr[wnk]] = nodeid[k].ravel()
    # sanity: rank must fit
    assert (rank < csarr[wn]).all()

    # weight-derived constants
    W_node = np.asarray(W_node, np.float32)
    att = np.asarray(att, np.float32)
    Wn3 = W_node.reshape(H, F, F)
    att_l = att[:, :F]
    att_r = att[:, F:]
    Ael = np.einsum('hfg,hf->gh', Wn3, att_l).astype(np.float32)
    Aer = np.einsum('hfg,hf->gh', Wn3, att_r).astype(np.float32)
    wcomb = np.concatenate([W_node.T, Ael, Aer], axis=1)      # [128, 390]

    b_node = np.asarray(b_node, np.float32)
    bias = np.asarray(bias, np.float32)
    ln_gamma = np.asarray(ln_gamma, np.float32)
    ln_beta = np.asarray(ln_beta, np.float32)
    add_node_bias = bool(np.any(b_node != 0))
    add_out_bias = bool(np.any(bias != 0))
    gamma_one = bool(np.all(ln_gamma == 1))
    beta_zero = bool(np.all(ln_beta == 0))

    hp = np.zeros((NP, F), np.float32)
    hp[:N_NODES] = h

    common = {
        "hT": np.ascontiguousarray(hp.T).astype(ml_dtypes.bfloat16),
        "wcomb": wcomb.astype(ml_dtypes.bfloat16),
        "wsc": np.asarray(w_scale, np.float32).astype(ml_dtypes.bfloat16),
        "ident": np.eye(128, dtype=np.float32),
        "padc": _pad_row_const(),
    }
    if add_node_bias:
        cel = (b_node.reshape(H, F) * att_l).sum(1)
        cer = (b_node.reshape(H, F) * att_r).sum(1)
        browv = np.concatenate([b_node, cel, cer]).astype(np.float32)
        common["brow"] = np.tile(browv[None, :], (128, 1))
    if not gamma_one:
        common["gam"] = np.tile(ln_gamma[None, :], (128, 1)).astype(np.float32)
    if not beta_zero:
        common["bet"] = np.tile(ln_beta[None, :], (128, 1)).astype(np.float32)
    if add_out_bias:
        common["bia"] = np.tile(bias[None, :], (128, 1)).astype(np.float32)

    in_maps = []
    for k in range(NCORES):
        m = dict(common)
        m["wint"] = np.ascontiguousarray(wint[k])
        m["hwin"] = np.ascontiguousarray(hp[nodeid[k].ravel()])
        in_maps.append(m)
    flags = (add_node_bias, add_out_bias, gamma_one, beta_zero)
    return CS, SI_TOT, flags, in_maps, nodeid


def kernel(h, src, dst, W_node, b_node, att, w_scale, bias, ln_gamma, ln_beta,
           _want_trace=False):
    CS, SI_TOT, flags, in_maps, nodeid = _host_prep(
        h, src, dst, W_node, b_node, att, w_scale, bias, ln_gamma, ln_beta)
    key = (CS, flags)
    if key not in _PROGRAM_CACHE:
        _PROGRAM_CACHE[key] = _build_program(CS, SI_TOT, *flags)
    nc = _PROGRAM_CACHE[key]
    res = run_bass_kernel_spmd(nc, in_maps, list(range(NCORES)), trace=_want_trace)
    out = np.zeros((N_NODES, F), np.float32)
    for k in range(NCORES):
        rows = res.results[k]["outy"]          # [NPC, F]
        ids = nodeid[k].ravel()
        msk = ids < N_NODES
        out[ids[msk]] = rows[msk]
    if _want_trace:
        kernel._last_exec_time_ns = res.exec_time_ns
        kernel._last_trace = res.instructions_and_trace
    return out

